# revision 1
# baseline (speedup 1.0000x reference)
"""Trainium2 Bass kernel for the ClassifierModel IoU-match loss.

Strategy: data-parallel over the batch axis B across 8 NeuronCores
(16 images per core). Inside each core the per-image [L=128, P=4096]
IoU/argmax matching runs in 32 chunks of [128 proposals x 128 labels],
fully fused on-chip.

The per-chunk elementwise pipeline uses three custom DVE ops
(registered at import via the documented dve_ops.OPS mechanism; the
micro-op programs are compiled into the per-NEFF DVE table, so they
execute natively on hardware):

  IV_OVERLAP_ANT      v  = min(LYH, ryh) - max(LY, ry)         (1 op)
  IV_RELU_ANT         ur = relu(min(LXW, rxw) - max(LX, rx))   (1 op)
  PROX_RECIP_MAX_ANT  prox = inter * ~recip(LA + ra)           (1 op)
                      accum_out = rowmax(prox)
    (~recip = BITWISE_NOT exponent-flip seed + one Chebyshev-scaled
     Newton pass, ~0.4% rel err -- only the argmax ordering consumes
     it, so near-ties may flip; the loss impact is far below the 2e-2
     gate.)

  inter = ur * v runs on the Pool engine (tensor_tensor mult) and the
  argmax one-hot mask (is_equal vs the row max) on Pool tensor_scalar,
  so DVE / Pool / ACT / PE all stay busy:

    DVE  : ur, v, prox+rowmax           (3 ops/chunk)
    Pool : inter, mask                  (2 ops/chunk)
    PE   : mask transpose + one-hot gather matmul vs [lx ly lw lh idx]
    ACT  : PSUM->SBUF mask copies, batched 4 chunks per copy

  Bbox targets, Huber, and the softmax cross-entropy terms are
  evaluated on wide [128, 512] tiles covering all 16 images at once,
  reduced to a single scalar per core; host sums the 8 core scalars.
"""

import sys

import numpy as np

sys.path.insert(0, "/opt/trn_rl_repo")

B, P, L = 128, 4096, 128
NCORES = 8
IMG = B // NCORES            # images per core
C = P // 128                 # chunks (free columns) per image
GRP = 4                      # chunks per mask-transpose copy group
SCALE = 32.0

LOG01 = float(np.log(np.float32(0.1)))
LOG09 = float(np.log(np.float32(0.9)))
CE_SLOPE = LOG01 - LOG09     # ~ -2.1972246


def _register_custom_ops():
    """Register the three fused DVE ops in concourse.dve_ops.OPS.

    Idempotent; uses the documented extension point (append to OPS with
    computed uops_sha pins). Rows stay below the 5-bit limit."""
    from concourse.dve_ops import (DveOp, OPS, CUSTOM_DVE_SPECS,
                                   _SUB_OPCODE_FOR_NAME, _CUSTOM_DVE_ROW_BASE,
                                   RECIP_APPROX_FAST_CONSTS)
    from concourse.dve_spec import (Spec, Src0, Src1, C0, C1, C2, Bin, AluOp,
                                    minn, maxx, relu, lower)
    from concourse.dve_uop import DveOpSpec

    cs = RECIP_APPROX_FAST_CONSTS

    def reg(name, spec, rd1):
        if name in _SUB_OPCODE_FOR_NAME:
            return next(o for o in OPS if o.name == name)
        row = _CUSTOM_DVE_ROW_BASE + len(OPS)
        assert row < 0x20
        shas = {}
        for ver in ("v3", "v4"):
            tmp = DveOpSpec(name=name, opcode=row,
                            uops=lower(spec, ver=ver), rd1_en=rd1)
            shas[ver] = tmp.sha(ver)
        op = DveOp(name, spec, subdim=False, uops_sha=shas)
        OPS.append(op)
        CUSTOM_DVE_SPECS[name] = spec
        _SUB_OPCODE_FOR_NAME[name] = row
        return op

    def np_recip1(y, s1, s2):
        ny = (~np.ascontiguousarray(y, np.float32).view(np.int32))
        ny = ny.view(np.float32)
        y0 = ny * np.float32(s1)
        return y0 * (np.float32(s2) - y * y0)

    iv = reg("IV_OVERLAP_ANT", Spec(
        body=minn(Src0, C0) - maxx(Src1, C1),
        reference=lambda in0, in1, s0, s1, imm2: (
            np.minimum(in0.astype(np.float32), s0)
            - np.maximum(in1.astype(np.float32), s1)),
    ), rd1=True)

    ivr = reg("IV_RELU_ANT", Spec(
        body=relu(minn(Src0, C0) - maxx(Src1, C1)),
        reference=lambda in0, in1, s0, s1, imm2: np.maximum(
            np.nan_to_num(np.minimum(in0.astype(np.float32), s0)
                          - np.maximum(in1.astype(np.float32), s1)), 0.0),
    ), rd1=True)

    _y = Src1 + C0
    _ny = Bin(AluOp.BITWISE_NOT, _y, _y)
    _y0 = _ny * C1
    _y1 = _y0 * (C2 - _y * _y0)

    def prm_ref(in0, in1, s0, s1, imm2):
        r = np_recip1(in1.astype(np.float32) + s0, s1, imm2)
        b = (in0.astype(np.float32) * r).astype(np.float32)
        b2 = b.reshape(b.shape[0], -1)
        return b, np.max(b2, axis=-1, keepdims=True)

    prm = reg("PROX_RECIP_MAX_ANT", Spec(
        body=Src0 * _y1, accum=AluOp.MAX, reference=prm_ref,
    ), rd1=True)

    return iv, ivr, prm, cs


def build(img=IMG):
    """Build + compile the per-core Bass program. Returns the Bacc."""
    from contextlib import ExitStack

    import concourse.tile as tile
    from concourse import bacc, mybir

    IV, IVR, PRM, CS = _register_custom_ops()

    f32 = mybir.dt.float32
    Alu = mybir.AluOpType
    Act = mybir.ActivationFunctionType
    X = mybir.AxisListType.X

    W = img * C              # wide column count (img=16 -> 512)
    nc = bacc.Bacc("TRN2", target_bir_lowering=False, debug=False,
                   enable_asserts=True, num_devices=NCORES)

    cls_d = nc.dram_tensor("cls", [img, 2 * P], f32, kind="ExternalInput").ap()
    bbox_d = nc.dram_tensor("bbox", [img, 4 * P], f32, kind="ExternalInput").ap()
    roi_d = nc.dram_tensor("roi", [img, P, 4], f32, kind="ExternalInput").ap()
    lab_d = nc.dram_tensor("labels", [img, L, 4], f32, kind="ExternalInput").ap()
    ident_d = nc.dram_tensor("ident", [128, 128], f32, kind="ExternalInput").ap()
    ones128_d = nc.dram_tensor("ones128", [128, 128], f32,
                               kind="ExternalInput").ap()
    iotap_d = nc.dram_tensor("iotap", [128, 1], f32, kind="ExternalInput").ap()
    ngate_d = nc.dram_tensor("ngate", [128, 1], f32, kind="ExternalInput").ap()
    onesr_d = nc.dram_tensor("onesr", [1, 128], f32, kind="ExternalInput").ap()
    onesc_d = nc.dram_tensor("onesc", [128, 1], f32, kind="ExternalInput").ap()
    out_d = nc.dram_tensor("out", [1, 1], f32, kind="ExternalOutput").ap()

    with tile.TileContext(nc) as tc, ExitStack() as ctx:
        cpool = ctx.enter_context(tc.tile_pool(name="consts", bufs=1))
        wpool = ctx.enter_context(tc.tile_pool(name="wide", bufs=1))
        ipool = ctx.enter_context(tc.tile_pool(name="perimg", bufs=2))
        kpool = ctx.enter_context(tc.tile_pool(name="chunk", bufs=6))
        mpool = ctx.enter_context(tc.tile_pool(name="mskt", bufs=3))
        pT = ctx.enter_context(tc.tile_pool(name="ptrans", bufs=2, space="PSUM"))
        pM = ctx.enter_context(tc.tile_pool(name="pmatch", bufs=2, space="PSUM"))
        pB = ctx.enter_context(tc.tile_pool(name="pbcast", bufs=1, space="PSUM"))
        pS = ctx.enter_context(tc.tile_pool(name="psmall", bufs=1, space="PSUM"))

        # ---- constants
        ident = cpool.tile([128, 128], f32)
        nc.sync.dma_start(ident[:], ident_d[:])
        ones128 = cpool.tile([128, 128], f32)
        nc.sync.dma_start(ones128[:], ones128_d[:])
        iotap = cpool.tile([128, 1], f32)
        nc.sync.dma_start(iotap[:], iotap_d[:])
        ngate = cpool.tile([128, 1], f32)
        nc.sync.dma_start(ngate[:], ngate_d[:])
        onesr = cpool.tile([1, 128], f32)
        nc.sync.dma_start(onesr[:], onesr_d[:])
        onesc = cpool.tile([128, 1], f32)
        nc.sync.dma_start(onesc[:], onesc_d[:])

        # ---- per-core wide tiles (col j = i*C + c; proposal g = p*C + c)
        CLS0 = wpool.tile([128, W], f32)
        CLS1 = wpool.tile([128, W], f32)
        PRED = wpool.tile([128, 4 * W], f32)   # col = k*W + j
        RXYWH = wpool.tile([128, 4 * W], f32)  # col = k*W + j (scaled roi)
        RA = wpool.tile([128, W], f32)
        RXW = wpool.tile([128, W], f32)
        RYH = wpool.tile([128, W], f32)
        MX = wpool.tile([128, W], f32)
        MATCH = wpool.tile([128, 5 * W], f32)  # col = j*5 + k

        def rxs(sl):
            return RXYWH[:, 0 * W:1 * W][:, sl]

        def rys(sl):
            return RXYWH[:, 1 * W:2 * W][:, sl]

        def rws(sl):
            return RXYWH[:, 2 * W:3 * W][:, sl]

        def rhs_(sl):
            return RXYWH[:, 3 * W:4 * W][:, sl]

        for i in range(img):
            jsl = slice(i * C, (i + 1) * C)

            # ---- loads (contiguous rows per partition)
            ROI = ipool.tile([128, 4 * C], f32, tag="roi")   # (c,k) interleaved
            nc.sync.dma_start(
                ROI[:], roi_d[i].rearrange("(p c) k -> p (c k)", c=C))
            nc.sync.dma_start(
                CLS0[:, jsl], cls_d[i, 0:P].rearrange("(p c) -> p c", c=C))
            nc.sync.dma_start(
                CLS1[:, jsl], cls_d[i, P:2 * P].rearrange("(p c) -> p c", c=C))
            for k in range(4):
                nc.sync.dma_start(
                    PRED[:, k * W + i * C: k * W + (i + 1) * C],
                    bbox_d[i, k * P:(k + 1) * P].rearrange("(p c) -> p c", c=C))

            # ---- per-proposal scalars: one strided [128, (4,C)] scale-copy
            rv = ROI[:].rearrange("p (c k) -> p k c", k=4)
            rxy_view = RXYWH[:].rearrange("p (k w) -> p k w",
                                          k=4)[:, :, i * C:(i + 1) * C]
            nc.scalar.activation(rxy_view, rv, Act.Copy, scale=SCALE)
            nc.vector.tensor_tensor(RA[:, jsl], rws(jsl), rhs_(jsl), Alu.mult)
            nc.vector.tensor_tensor(RXW[:, jsl], rxs(jsl), rws(jsl), Alu.add)
            nc.vector.tensor_tensor(RYH[:, jsl], rys(jsl), rhs_(jsl), Alu.add)

            # ---- label rows -> broadcast tiles [128, 640]
            LROW = ipool.tile([1, 640], f32, tag="lrow")
            nc.sync.dma_start(
                LROW[0:1, 0:512].rearrange("p (k l) -> p k l", k=4),
                lab_d[i].rearrange("l k -> k l")[None])
            nc.vector.tensor_tensor(LROW[0:1, 512:640], LROW[0:1, 256:384],
                                    LROW[0:1, 384:512], Alu.mult)  # la = lw*lh
            lxr = LROW[0:1, 0:128]
            lyr = LROW[0:1, 128:256]
            lwr = LROW[0:1, 256:384]
            lhr = LROW[0:1, 384:512]
            lar = LROW[0:1, 512:640]
            # ---- gather rhs: [lx, ly, lw, lh, label_index]
            LAB5 = ipool.tile([128, 5], f32, tag="lab5")
            nc.sync.dma_start(LAB5[:, 0:4], lab_d[i])
            nc.scalar.copy(LAB5[:, 4:5], iotap[:])

            BCp = pB.tile([128, 648], f32, tag="bcp")
            # blocks: LX, LXW, LY, LYH, LA, then labsum5 (for the
            # complement-gather correction)
            nc.tensor.matmul(BCp[:, 640:645], ones128[:], LAB5[:],
                             start=True, stop=True)
            nc.tensor.matmul(BCp[:, 0:128], onesr[:], lxr, start=True, stop=True)
            nc.tensor.matmul(BCp[:, 128:256], onesr[:], lxr,
                             start=True, stop=False)
            nc.tensor.matmul(BCp[:, 128:256], onesr[:], lwr,
                             start=False, stop=True)
            nc.tensor.matmul(BCp[:, 256:384], onesr[:], lyr,
                             start=True, stop=True)
            nc.tensor.matmul(BCp[:, 384:512], onesr[:], lyr,
                             start=True, stop=False)
            nc.tensor.matmul(BCp[:, 384:512], onesr[:], lhr,
                             start=False, stop=True)
            nc.tensor.matmul(BCp[:, 512:640], onesr[:], lar,
                             start=True, stop=True)
            BC = ipool.tile([128, 648], f32, tag="bc")
            nc.scalar.copy(BC[:], BCp[:])
            LX, LXW = BC[:, 0:128], BC[:, 128:256]
            LY, LYH = BC[:, 256:384], BC[:, 384:512]
            LA = BC[:, 512:640]
            LS5 = BC[:, 640:645]

            MPS = pM.tile([128, 5 * C], f32, tag="mps")
            for g in range(C // GRP):
                TB = pT.tile([128, 128 * GRP], f32, tag="tb")
                for q in range(GRP):
                    c = g * GRP + q
                    j = i * C + c
                    jj = slice(j, j + 1)

                    ur = kpool.tile([128, 128], f32, tag="ur")
                    nc.vector._custom_dve(
                        IVR, out=ur[:], in0=LXW, in1=LX,
                        s0=RXW[:, jj], s1=rxs(jj))
                    v = kpool.tile([128, 128], f32, tag="v")
                    nc.vector._custom_dve(
                        IV, out=v[:], in0=LYH, in1=LY,
                        s0=RYH[:, jj], s1=rys(jj))
                    inter = kpool.tile([128, 128], f32, tag="inter")
                    nc.gpsimd.tensor_tensor(inter[:], ur[:], v[:], Alu.mult)
                    prox = kpool.tile([128, 128], f32, tag="prox")
                    nc.vector._custom_dve(
                        PRM, out=prox[:], accum_out=MX[:, jj], in0=inter[:],
                        in1=LA, s0=RA[:, jj], s1=CS["s0"], imm2=CS["s1"])
                    # complement one-hot: sign(MX - prox) = 0 at the
                    # winner, 1 elsewhere (runs on the ACT engine)
                    msk = kpool.tile([128, 128], f32, tag="msk")
                    nc.scalar.activation(msk[:], prox[:], Act.Sign,
                                         bias=MX[:, jj], scale=-1.0)
                    nc.tensor.transpose(TB[:, q * 128:(q + 1) * 128], msk[:],
                                        ident[:])
                MSKT = mpool.tile([128, 128 * GRP], f32, tag="mskt")
                nc.scalar.copy(MSKT[:], TB[:])
                for q in range(GRP):
                    c = g * GRP + q
                    nc.tensor.matmul(MPS[:, c * 5:(c + 1) * 5],
                                     MSKT[:, q * 128:(q + 1) * 128], LAB5[:],
                                     start=True, stop=True)
            # matched = labsum - complement_gather (one DVE sub, MPS in PSUM)
            ls_bc = LS5.rearrange("p (o k) -> p o k", o=1).broadcast_to(
                [128, C, 5])
            mview3 = MATCH[:, i * 5 * C:(i + 1) * 5 * C].rearrange(
                "p (c k) -> p c k", k=5)
            mps3 = MPS[:].rearrange("p (c k) -> p c k", k=5)
            nc.vector.tensor_tensor(mview3, ls_bc, mps3, Alu.subtract)

        # ---- wide per-proposal loss stage, col j = (i, c)
        wa = slice(0, W)

        def mview(k):
            return MATCH[:].rearrange("p (j k) -> p k j", k=5)[:, k]

        RCPW = wpool.tile([128, W], f32)
        nc.vector.reciprocal_approx_fast(RCPW[:], rws(wa))
        RCPH = wpool.tile([128, W], f32)
        nc.vector.reciprocal_approx_fast(RCPH[:], rhs_(wa))

        pidx = wpool.tile([128, W], f32)
        nc.vector.tensor_scalar(pidx[:], mview(4), 0.5, None, Alu.is_gt)
        hit = wpool.tile([128, W], f32)
        nc.vector.tensor_scalar(hit[:], MX[:], 0.0, None, Alu.is_gt)
        pos = wpool.tile([128, W], f32)
        nc.vector.tensor_tensor(pos[:], hit[:], pidx[:], Alu.mult)

        T4 = wpool.tile([128, 4 * W], f32)
        tmp = wpool.tile([128, W], f32)
        # tx, ty
        nc.vector.tensor_tensor(tmp[:], mview(0), rxs(wa), Alu.subtract)
        nc.vector.tensor_tensor(T4[:, 0:W], tmp[:], RCPW[:], Alu.mult)
        tmp2 = wpool.tile([128, W], f32)
        nc.vector.tensor_tensor(tmp2[:], mview(1), rys(wa), Alu.subtract)
        nc.vector.tensor_tensor(T4[:, W:2 * W], tmp2[:], RCPH[:], Alu.mult)
        # tw, th (safe log)
        qw = wpool.tile([128, W], f32)
        nc.vector.tensor_tensor(qw[:], mview(2), RCPW[:], Alu.mult)
        nc.vector.tensor_scalar(qw[:], qw[:], 1e-8, None, Alu.max)
        nc.scalar.activation(T4[:, 2 * W:3 * W], qw[:], Act.Ln)
        qh = wpool.tile([128, W], f32)
        nc.vector.tensor_tensor(qh[:], mview(3), RCPH[:], Alu.mult)
        nc.vector.tensor_scalar(qh[:], qh[:], 1e-8, None, Alu.max)
        nc.scalar.activation(T4[:, 3 * W:4 * W], qh[:], Act.Ln)

        # Huber over the packed [128, 4W] tiles
        ERR = wpool.tile([128, 4 * W], f32)
        nc.vector.tensor_tensor(ERR[:], T4[:], PRED[:], Alu.subtract)
        AE = wpool.tile([128, 4 * W], f32)
        nc.scalar.activation(AE[:], ERR[:], Act.Abs)
        M_ = wpool.tile([128, 4 * W], f32)
        nc.vector.tensor_scalar(M_[:], AE[:], 1.0, None, Alu.min)
        U1 = wpool.tile([128, 4 * W], f32)
        nc.vector.tensor_scalar(U1[:], M_[:], 0.5, -1.0, Alu.mult, Alu.add)
        V1 = wpool.tile([128, 4 * W], f32)
        nc.vector.tensor_tensor(V1[:], U1[:], M_[:], Alu.mult)
        H4 = wpool.tile([128, 4 * W], f32)
        nc.vector.tensor_tensor(H4[:], V1[:], AE[:], Alu.add)
        hs01 = wpool.tile([128, W], f32)
        nc.vector.tensor_tensor(hs01[:], H4[:, 0:W], H4[:, W:2 * W], Alu.add)
        hs23 = wpool.tile([128, W], f32)
        nc.vector.tensor_tensor(hs23[:], H4[:, 2 * W:3 * W], H4[:, 3 * W:4 * W],
                                Alu.add)
        hsum = wpool.tile([128, W], f32)
        nc.vector.tensor_tensor(hsum[:], hs01[:], hs23[:], Alu.add)

        # classification terms from sig = softmax prob of class 1
        dlog = wpool.tile([128, W], f32)
        nc.vector.tensor_tensor(dlog[:], CLS1[:], CLS0[:], Alu.subtract)
        sig = wpool.tile([128, W], f32)
        nc.scalar.activation(sig[:], dlog[:], Act.Sigmoid)
        cepos = wpool.tile([128, W], f32)
        nc.vector.tensor_scalar(cepos[:], sig[:], CE_SLOPE, -LOG01,
                                Alu.mult, Alu.add)
        negt = wpool.tile([128, W], f32)
        nc.vector.tensor_scalar(negt[:], sig[:], -CE_SLOPE, -LOG09,
                                Alu.mult, Alu.add)
        nc.vector.tensor_scalar(negt[:], negt[:], ngate[:, 0:1], None, Alu.mult)

        # per_prop = pos * (0.5*hsum + cepos - negt) + negt
        inner = wpool.tile([128, W], f32)
        nc.vector.tensor_scalar(inner[:], hsum[:], 0.5, None, Alu.mult)
        nc.vector.tensor_tensor(inner[:], inner[:], cepos[:], Alu.add)
        nc.vector.tensor_tensor(inner[:], inner[:], negt[:], Alu.subtract)
        nc.vector.tensor_tensor(inner[:], inner[:], pos[:], Alu.mult)
        nc.vector.tensor_tensor(inner[:], inner[:], negt[:], Alu.add)

        PPR = wpool.tile([128, 1], f32)
        nc.vector.tensor_reduce(PPR[:], inner[:], X, Alu.add)
        FS = pS.tile([1, 1], f32, tag="fs")
        nc.tensor.matmul(FS[:], PPR[:], onesc[:], start=True, stop=True)
        fsb = cpool.tile([1, 1], f32)
        nc.scalar.copy(fsb[:], FS[:])
        nc.sync.dma_start(out_d[:], fsb[:])

    nc.compile()
    return nc


_CACHE = {}


def _get_program(img=IMG):
    key = img
    if key not in _CACHE:
        _CACHE[key] = build(img)
    return _CACHE[key]


def _make_in_maps(np_inputs):
    cls_scores = np.ascontiguousarray(np_inputs["cls_scores"], dtype=np.float32)
    bbox_deltas = np.ascontiguousarray(np_inputs["bbox_deltas"],
                                       dtype=np.float32)
    roi = np.ascontiguousarray(np_inputs["roi"], dtype=np.float32)
    labels = np.ascontiguousarray(np_inputs["labels"], dtype=np.float32)
    ng = np.float32(
        1.0 if int(np.asarray(np_inputs["neg_enabled"])) > 0 else 0.0)

    ident = np.eye(128, dtype=np.float32)
    ones128 = np.ones((128, 128), dtype=np.float32)
    iotap = np.arange(128, dtype=np.float32).reshape(128, 1)
    ngate = np.full((128, 1), ng, dtype=np.float32)
    onesr = np.ones((1, 128), dtype=np.float32)
    onesc = np.ones((128, 1), dtype=np.float32)

    in_maps = []
    for core in range(NCORES):
        sl = slice(core * IMG, (core + 1) * IMG)
        in_maps.append({
            "cls": cls_scores[sl],
            "bbox": bbox_deltas[sl],
            "roi": roi[sl],
            "labels": labels[sl],
            "ident": ident,
            "ones128": ones128,
            "iotap": iotap,
            "ngate": ngate,
            "onesr": onesr,
            "onesc": onesc,
        })
    return in_maps


def kernel(cls_scores, bbox_deltas, roi, labels, neg_enabled):
    from concourse.bass_utils import run_bass_kernel_spmd

    nc = _get_program()
    in_maps = _make_in_maps({
        "cls_scores": cls_scores,
        "bbox_deltas": bbox_deltas,
        "roi": roi,
        "labels": labels,
        "neg_enabled": neg_enabled,
    })
    res = run_bass_kernel_spmd(nc, in_maps, list(range(NCORES)))
    total = np.float32(0.0)
    for r in res.results:
        total += np.float32(r["out"][0, 0])
    return np.float32(total)

